# revision 1
# baseline (speedup 1.0000x reference)
import sys as _sys
import os as _os

for _p in ("/opt/trn_rl_repo", _os.path.expanduser("~/.axon_site/_ro/trn_rl_repo")):
    if _os.path.isdir(_p) and _p not in _sys.path:
        _sys.path.append(_p)

"""Builder for the sliding-window attention kernel (NaiveHybridAttention).

Per-core program (SPMD, head-sharded):
  inputs (per core): xT (B,D,S), wqT/wkT/wvT (D,E), woT (E,D),
                     cos/sin RoPE tables (HD,S), additive masks (128,768)
  output: part (B,S,D) = this core's heads' contribution to the final
          out-projection; host sums the 8 partials.

Pipeline per batch:
  A) QKV: qT,kT = W^T-stationary matmuls -> [e, S] (RoPE fused into PSUM
     evacuation, scale folded into q tables); v = x-stationary -> [s, e].
  B) Attention per head: scores [q,k] in 256-query blocks over a <=768
     key span, additive window mask, exp (+row-sum via accum_out) on ACT,
     normalize, PE-transpose probs, av accumulates attnT [hd, q].
  C) Out-proj: attnT-stationary -> psum [s, o] -> DMA to part.

All matmuls run as float32r (full fp32 storage; 1 cycle/row at N>=256).
PSUM lives in one pool with 8 explicitly-tagged bank-sized slots shared
across phases (T1..T8).
"""

import os

import numpy as np
import concourse.bass as bass
from concourse import mybir

USE_GPSIMD_ADD = os.environ.get("NHA_GPSIMD_ADD", "0") == "1"
USE_TTR = os.environ.get("NHA_TTR", "0") == "1"
USE_F32R = os.environ.get("NHA_F32R", "1") == "1"

F32 = mybir.dt.float32
F32R = mybir.dt.float32r if USE_F32R else mybir.dt.float32
ROPE_BASE = 10000.0
WINDOW = 512
MASKW = 768
NEG = -10000.0


def r32(ap):
    return ap.bitcast(F32R) if USE_F32R else ap


def host_tables(S, HD=128):
    """cos/sin tables in transposed layout [HD, S]; sin is sign-folded so
    q_rope = q*cos_t + swap_halves(q)*sin_sg. Unscaled — the softmax 1/sqrt(HD)
    is applied via the Exp activation's scale parameter."""
    inv_freq = 1.0 / (ROPE_BASE ** (np.arange(0, HD, 2, dtype=np.float64) / HD))
    fr = np.arange(S, dtype=np.float64)[None, :] * inv_freq[:, None]  # [HD/2, S]
    cos = np.cos(fr)
    sin = np.sin(fr)
    cos_t = np.concatenate([cos, cos], 0).astype(np.float32)
    sin_sg = np.concatenate([-sin, sin], 0).astype(np.float32)
    return cos_t, sin_sg


def host_masks():
    """Multiplicative (1.0 valid / 0.0 invalid) sliding-window masks, applied
    to exp(scores) on the DVE (fused with the row-sum)."""
    r = np.arange(128)[:, None]
    c = np.arange(MASKW)[None, :]
    maskA = ((c >= r + 1) & (c <= r + 512)).astype(np.float32)
    maskB = ((c >= r + 129) & (c <= r + 640)).astype(np.float32)
    return maskA, maskB


def partial_ref_np(x, wq_r, wk_r, wv_r, wo_t):
    """NumPy mirror of the per-core computation (fp32).
    x: (B,S,D); wq_r/wk_r/wv_r: (E,D) row-slices of w_qkv; wo_t: (E,D) =
    w_out[:, e_slice].T. Returns (B,S,D) partial."""
    B, S, D = x.shape
    E = wq_r.shape[0]
    HC = E // 128
    q = np.einsum("bsd,ed->bse", x, wq_r).reshape(B, S, HC, 128)
    k = np.einsum("bsd,ed->bse", x, wk_r).reshape(B, S, HC, 128)
    v = np.einsum("bsd,ed->bse", x, wv_r).reshape(B, S, HC, 128)
    inv_freq = 1.0 / (ROPE_BASE ** (np.arange(0, 128, 2, dtype=np.float64) / 128))
    fr = np.arange(S, dtype=np.float64)[:, None] * inv_freq[None, :]
    emb = np.concatenate([fr, fr], -1)
    cos = np.cos(emb).astype(np.float32)[None, :, None, :]
    sin = np.sin(emb).astype(np.float32)[None, :, None, :]

    def rot(t):
        t1, t2 = t[..., :64], t[..., 64:]
        return np.concatenate([-t2, t1], -1)

    q = q * cos + rot(q) * sin
    k = k * cos + rot(k) * sin
    scale = 1.0 / np.sqrt(128.0)
    i = np.arange(S)[:, None]
    j = np.arange(S)[None, :]
    valid = (i - j >= 0) & (i - j < WINDOW)
    out = np.zeros((B, S, E), np.float32)
    for b in range(B):
        for h in range(HC):
            s = (q[b, :, h] @ k[b, :, h].T) * scale
            s = np.where(valid, s, -np.inf)
            s = s - s.max(-1, keepdims=True)
            p = np.exp(s)
            p /= p.sum(-1, keepdims=True)
            out[b, :, h * 128 : (h + 1) * 128] = p @ v[b, :, h]
    return np.einsum("bse,ed->bsd", out, wo_t).astype(np.float32)


def declare_io(nc, B, S, D, E):
    dt = F32
    t = {}
    t["xt"] = nc.dram_tensor("xt", [B, D, S], dt, kind="ExternalInput").ap()
    for n in ("wqt", "wkt", "wvt"):
        t[n] = nc.dram_tensor(n, [D, E], dt, kind="ExternalInput").ap()
    t["wot"] = nc.dram_tensor("wot", [E, D], dt, kind="ExternalInput").ap()
    for n in ("cost", "sint"):
        t[n] = nc.dram_tensor(n, [128, S], dt, kind="ExternalInput").ap()
    t["maskA"] = nc.dram_tensor("maskA", [128, MASKW], dt, kind="ExternalInput").ap()
    t["maskB"] = nc.dram_tensor("maskB", [128, MASKW], dt, kind="ExternalInput").ap()
    t["part"] = nc.dram_tensor("part", [B, S, D], dt, kind="ExternalOutput").ap()
    return t


def build_program(ctx, nc, tc, io, B, S, D, HC, reps=1):
    """Emit the per-core program. HC = heads on this core; E = HC*128.
    reps > 1 wraps the body in a hardware loop repeating the identical
    computation (for timing measurements); output is unchanged."""
    E = HC * 128
    KT = D // 128  # contraction tiles for qkv
    SC = S // 512  # s-chunks for qkv
    QB = S // 256  # query blocks for attention
    ST = S // 128
    OCW = min(512, D)
    OC = D // OCW

    const = ctx.enter_context(tc.tile_pool(name="const", bufs=1))
    work = ctx.enter_context(tc.tile_pool(name="work", bufs=1))
    xsp = ctx.enter_context(tc.tile_pool(name="xs", bufs=5))
    tmp = ctx.enter_context(tc.tile_pool(name="tmp", bufs=2))
    smp = ctx.enter_context(tc.tile_pool(name="sm", bufs=1))
    pp = ctx.enter_context(tc.tile_pool(name="pp", bufs=2))
    rp = ctx.enter_context(tc.tile_pool(name="rp", bufs=4))
    ptp = ctx.enter_context(tc.tile_pool(name="pt", bufs=1))
    outp = ctx.enter_context(tc.tile_pool(name="outp", bufs=2))
    ps = ctx.enter_context(tc.tile_pool(name="ps", bufs=1, space="PSUM"))

    # ---- constants ----
    # q/k/v weights: one DMA per 128-row k-tile so the first matmuls only
    # depend on the slices they read (kills the startup stall). Other consts
    # go on the gpsimd (SWDGE) queue to stay off the HWDGE queue that
    # streams x.
    wq_sb = const.tile([128, KT, E], F32R)
    wk_sb = const.tile([128, KT, E], F32R)
    wv_sb = const.tile([128, KT, E], F32R)
    for kt in range(KT):
        rows = bass.ts(kt, 128)
        nc.gpsimd.dma_start(wq_sb[:, kt, :], r32(io["wqt"][rows, :]))
        nc.gpsimd.dma_start(wk_sb[:, kt, :], r32(io["wkt"][rows, :]))
        nc.gpsimd.dma_start(wv_sb[:, kt, :], r32(io["wvt"][rows, :]))
    wo_sb = const.tile([128, HC, D], F32R)
    nc.gpsimd.dma_start(wo_sb[:], r32(io["wot"].rearrange("(et p) o -> p et o", p=128)))
    cost = const.tile([128, S], F32)
    nc.gpsimd.dma_start(cost[:], io["cost"][:])
    sint = const.tile([128, S], F32)
    nc.gpsimd.dma_start(sint[:], io["sint"][:])
    mA = const.tile([128, MASKW], F32)
    nc.gpsimd.dma_start(mA[:], io["maskA"][:])
    mB = const.tile([128, MASKW], F32)
    nc.gpsimd.dma_start(mB[:], io["maskB"][:])
    ident = const.tile([128, 128], F32)
    from concourse.masks import make_identity

    make_identity(nc, ident[:])

    def rope(dst, src_ps, cos_t, sin_t, cols, w):
        """dst[:, cols] = src_ps*cos + swap_halves(src_ps)*sin (RoPE).
        Muls (PSUM readers) on DVE; final SBUF-only add on GpSimd to keep
        the DVE burst at chunk boundaries short."""
        rot = tmp.tile([128, 512], F32, tag="rot")
        nc.vector.tensor_mul(rot[0:64, :w], src_ps[64:128, :w], sin_t[0:64, cols])
        nc.vector.tensor_mul(rot[64:128, :w], src_ps[0:64, :w], sin_t[64:128, cols])
        cv = tmp.tile([128, 512], F32, tag="cosv")
        nc.vector.tensor_mul(cv[:, :w], src_ps[:, :w], cos_t[:, cols])
        if USE_GPSIMD_ADD:
            nc.gpsimd.tensor_add(dst, cv[:, :w], rot[:, :w])
        else:
            nc.vector.tensor_add(dst, cv[:, :w], rot[:, :w])

    def body():
        _emit_body(nc, tc, io, B, S, D, HC, locals_=dict(
            const=const, work=work, xsp=xsp, tmp=tmp, smp=smp, pp=pp, rp=rp,
            ptp=ptp, outp=outp, ps=ps,
            wq_sb=wq_sb, wk_sb=wk_sb, wv_sb=wv_sb, wo_sb=wo_sb,
            cost=cost, sint=sint, mA=mA, mB=mB, ident=ident, rope=rope,
        ))

    if reps > 1:
        with tc.For_i(0, reps, 1):
            body()
    else:
        body()


def _emit_body(nc, tc, io, B, S, D, HC, locals_):
    E = HC * 128
    KT = D // 128
    SC = S // 512
    QB = S // 256
    ST = S // 128
    OCW = min(512, D)
    OC = D // OCW
    const = locals_["const"]; work = locals_["work"]; xsp = locals_["xsp"]
    tmp = locals_["tmp"]; smp = locals_["smp"]; pp = locals_["pp"]
    rp = locals_["rp"]; ptp = locals_["ptp"]; outp = locals_["outp"]
    ps = locals_["ps"]
    wq_sb = locals_["wq_sb"]; wk_sb = locals_["wk_sb"]; wv_sb = locals_["wv_sb"]
    wo_sb = locals_["wo_sb"]; cost = locals_["cost"]; sint = locals_["sint"]
    mA = locals_["mA"]; mB = locals_["mB"]; ident = locals_["ident"]
    rope = locals_["rope"]

    for b in range(B):
        # ---- A) QKV projection ----
        qT = work.tile([128, HC, S], F32R, tag="qT")
        kT = work.tile([128, HC, S], F32R, tag="kT")
        v_sb = work.tile([128, ST, E], F32R, tag="v")
        for sc in range(SC):
            cols = bass.ts(sc, 512)
            q_ps = [
                ps.tile([128, 512], F32, tag=t, name=f"q_ps{i}")
                for i, t in enumerate(("T1", "T2")[:HC])
            ]
            k_ps = [
                ps.tile([128, 512], F32, tag=t, name=f"k_ps{i}")
                for i, t in enumerate(("T3", "T4")[:HC])
            ]
            v_ps = [
                ps.tile([128, E], F32, tag=t, name=f"v_ps{i}")
                for i, t in enumerate(("T5", "T6", "T7", "T8"))
            ]
            for kt in range(KT):
                xs = xsp.tile([128, 512], F32R)
                nc.sync.dma_start(xs[:], r32(io["xt"][b, bass.ts(kt, 128), cols]))
                f = dict(start=(kt == 0), stop=(kt == KT - 1))
                # v first: its psum slots are evacuated fastest, so the next
                # chunk's accumulation can begin while q/k RoPE evac runs
                for ss in range(4):
                    nc.tensor.matmul(
                        v_ps[ss][:],
                        xs[:, bass.ts(ss, 128)],
                        wv_sb[:, kt, :],
                        **f,
                    )
                for et in range(HC):
                    nc.tensor.matmul(
                        k_ps[et][:], wk_sb[:, kt, bass.ts(et, 128)], xs[:], **f
                    )
                    nc.tensor.matmul(
                        q_ps[et][:], wq_sb[:, kt, bass.ts(et, 128)], xs[:], **f
                    )
            for et in range(HC):
                rope(qT[:, et, cols], q_ps[et], cost, sint, cols, 512)
                rope(kT[:, et, cols], k_ps[et], cost, sint, cols, 512)
            for ss in range(4):
                nc.scalar.copy(v_sb[:, sc * 4 + ss, :], v_ps[ss][:])

        # ---- B) attention, per head ----
        # Pipelined per 256-query block: both subtiles' score matmuls are
        # issued before either softmax, so PE stays busy during the
        # mask+exp+normalize chain (in-order PE stream). Score psums use 4
        # tags (A/B piece x 2 subtiles); in-place mask-add in PSUM; exp
        # (+row-sum) reads PSUM directly on ACT.
        attnT = work.tile([128, HC, S], F32R, tag="attnT")
        exp_scale = float(1.0 / np.sqrt(128.0))

        def emit_qk(h, qb):
            """Score matmuls for both 128-query subtiles of block qb."""
            q0 = qb * 256
            kstart = max(0, q0 - WINDOW)
            kspan = q0 + 256 - kstart
            la = min(512, kspan)
            lb = kspan - la
            pieces = {}
            for sub in range(2):
                qcols = bass.ds(q0 + sub * 128, 128)
                spA = ps.tile(
                    [128, 512], F32, tag=("T1", "T2")[sub], name=f"spA{sub}"
                )
                nc.tensor.matmul(
                    spA[:, :la],
                    qT[:, h, qcols],
                    kT[:, h, bass.ds(kstart, la)],
                )
                pieces[sub] = [(spA, 0, la)]
                if lb:
                    spB = ps.tile(
                        [128, 256], F32, tag=("T5", "T6")[sub], name=f"spB{sub}"
                    )
                    nc.tensor.matmul(
                        spB[:, :lb],
                        qT[:, h, qcols],
                        kT[:, h, bass.ds(kstart + 512, lb)],
                    )
                    pieces[sub].append((spB, la, lb))
            return dict(h=h, q0=q0, kstart=kstart, kspan=kspan, pieces=pieces)

        def emit_rest(d):
            """Softmax + transposes + AV for a previously-issued block."""
            h, q0, kstart, kspan = d["h"], d["q0"], d["kstart"], d["kspan"]
            nkt = kspan // 128
            pTs = [
                ptp.tile([128, 256], F32R, tag=f"pT{i}", name=f"pT{i}")
                for i in range(nkt)
            ]
            p_all = {}
            for sub in range(2):
                msk = mA if sub == 0 else mB
                rsums = []
                p_sbs = []
                for pi, (sp, off, ln) in enumerate(d["pieces"][sub]):
                    # exp straight from PSUM (releases the score slot ASAP);
                    # window mask applied multiplicatively, fused with the
                    # row-sum, in one DVE op
                    p_sb = pp.tile(
                        [128, 512 if pi == 0 else 256],
                        F32,
                        tag=f"p{sub}{pi}",
                        name=f"p{sub}{pi}",
                    )
                    nc.scalar.activation(
                        p_sb[:, :ln],
                        sp[:, :ln],
                        mybir.ActivationFunctionType.Exp,
                        scale=exp_scale,
                    )
                    rs = rp.tile([128, 1], F32, tag=f"rs{sub}{pi}")
                    if USE_TTR:
                        nc.vector.tensor_tensor_reduce(
                            p_sb[:, :ln],
                            p_sb[:, :ln],
                            msk[:, bass.ds(MASKW - kspan + off, ln)],
                            1.0,
                            0.0,
                            mybir.AluOpType.mult,
                            mybir.AluOpType.add,
                            rs[:],
                        )
                    else:
                        nc.vector.tensor_mul(
                            p_sb[:, :ln],
                            p_sb[:, :ln],
                            msk[:, bass.ds(MASKW - kspan + off, ln)],
                        )
                        nc.vector.reduce_sum(
                            out=rs[:], in_=p_sb[:, :ln], axis=mybir.AxisListType.X
                        )
                    rsums.append(rs)
                    p_sbs.append((p_sb, off, ln))
                if len(rsums) == 2:
                    nc.vector.tensor_add(rsums[0][:], rsums[0][:], rsums[1][:])
                rinv = rp.tile([128, 1], F32, tag=f"rinv{sub}")
                nc.vector.reciprocal(rinv[:], rsums[0][:])
                for p_sb, off, ln in p_sbs:
                    nc.vector.tensor_scalar_mul(p_sb[:, :ln], p_sb[:, :ln], rinv[:])
                p_all[sub] = p_sbs
            for sub in range(2):
                for p_sb, off, ln in p_all[sub]:
                    for kk in range(ln // 128):
                        kt2 = (off + kk * 128) // 128
                        tp = ps.tile(
                            [128, 128],
                            F32,
                            tag=("T3" if kt2 % 2 == 0 else "T4"),
                            name=f"tp{kt2}",
                        )
                        nc.tensor.transpose(
                            tp[:],
                            p_sb[:, bass.ds(kk * 128, 128)],
                            ident[:],
                        )
                        nc.any.tensor_copy(pTs[kt2][:, bass.ts(sub, 128)], tp[:])
            av = ps.tile([128, 256], F32, tag="T7", name="av")
            for kt2 in range(nkt):
                nc.tensor.matmul(
                    av[:],
                    v_sb[:, kstart // 128 + kt2, bass.ts(h, 128)],
                    pTs[kt2][:],
                    start=(kt2 == 0),
                    stop=(kt2 == nkt - 1),
                )
            nc.any.tensor_copy(attnT[:, h, bass.ds(q0, 256)], av[:])

        # one-stage software pipeline: qk(i) is issued before the softmax/
        # transpose/AV of block i-1, so PE has work during the softmax chain
        prev = None
        for h in range(HC):
            for qb in range(QB):
                d = emit_qk(h, qb)
                if prev is not None:
                    emit_rest(prev)
                prev = d
        emit_rest(prev)

        # ---- C) out-projection (partial over this core's E dims) ----
        # evac copies alternate DVE/ACT; one batched 1 MB output DMA per
        # 128-row stripe instead of four 256 KB ones
        for st in range(ST):
            osb = outp.tile([128, D], F32, tag="osb")
            for oc in range(OC):
                o_ps = ps.tile(
                    [128, OCW], F32, tag=("T1" if oc % 2 == 0 else "T2"), name="o_ps"
                )
                for et in range(HC):
                    nc.tensor.matmul(
                        o_ps[:],
                        attnT[:, et, bass.ts(st, 128)],
                        wo_sb[:, et, bass.ts(oc, OCW)],
                        start=(et == 0),
                        stop=(et == HC - 1),
                    )
                dst = osb[:, bass.ts(oc, OCW)]
                if oc % 2 == 0:
                    nc.vector.tensor_copy(dst, o_ps[:])
                else:
                    nc.scalar.copy(dst, o_ps[:])
            nc.sync.dma_start(io["part"][b, bass.ts(st, 128), :], osb[:])


# ======================================================================
# 8-core SPMD wrapper
# ======================================================================
from contextlib import ExitStack as _ExitStack

N_CORES = 8
B_FULL, S_FULL, D_FULL, H_FULL, HD_FULL = 2, 2048, 2048, 16, 128
HC_FULL = H_FULL // N_CORES  # 2 heads per core

_nc_cache = {}


def get_compiled(reps=1):
    """Build + bacc-compile the per-core Bass program (cached per reps)."""
    if reps not in _nc_cache:
        import concourse.bacc as bacc
        from concourse import tile

        nc = bacc.Bacc(
            "TRN2", target_bir_lowering=False, debug=False, num_devices=N_CORES
        )
        io = declare_io(nc, B_FULL, S_FULL, D_FULL, HC_FULL * 128)
        with tile.TileContext(nc) as tc:
            with _ExitStack() as ctx:
                build_program(
                    ctx, nc, tc, io, B_FULL, S_FULL, D_FULL, HC_FULL, reps=reps
                )
        nc.compile()
        _nc_cache[reps] = nc
    return _nc_cache[reps]


def make_in_maps(x, w_qkv, w_out):
    """Host-side sharding: per-core input dicts (head-sharded)."""
    x = np.ascontiguousarray(np.asarray(x, dtype=np.float32))
    w_qkv = np.ascontiguousarray(np.asarray(w_qkv, dtype=np.float32))
    w_out = np.ascontiguousarray(np.asarray(w_out, dtype=np.float32))
    D = D_FULL
    xt = np.ascontiguousarray(x.transpose(0, 2, 1))
    cos_t, sin_t = host_tables(S_FULL)
    maskA, maskB = host_masks()
    in_maps = []
    for c in range(N_CORES):
        e0, e1 = c * HC_FULL * 128, (c + 1) * HC_FULL * 128
        in_maps.append(
            dict(
                xt=xt,
                wqt=np.ascontiguousarray(w_qkv[e0:e1].T),
                wkt=np.ascontiguousarray(w_qkv[D + e0 : D + e1].T),
                wvt=np.ascontiguousarray(w_qkv[2 * D + e0 : 2 * D + e1].T),
                wot=np.ascontiguousarray(w_out[:, e0:e1].T),
                cost=cos_t,
                sint=sin_t,
                maskA=maskA,
                maskB=maskB,
            )
        )
    return in_maps


def combine(parts):
    """Sum the 8 per-core out-projection partials."""
    acc = np.zeros((B_FULL, S_FULL, D_FULL), np.float64)
    for p in parts:
        acc += p
    return acc.astype(np.float32)


def kernel(x, w_qkv, w_out):
    from concourse import bass_utils

    nc = get_compiled(reps=1)
    in_maps = make_in_maps(x, w_qkv, w_out)
    res = bass_utils.run_bass_kernel_spmd(
        nc, in_maps, core_ids=list(range(N_CORES))
    )
    return combine([res.results[c]["part"] for c in range(N_CORES)])



# revision 14
# speedup vs baseline: 1.0776x; 1.0776x over previous
import sys as _sys
import os as _os

for _p in ("/opt/trn_rl_repo", _os.path.expanduser("~/.axon_site/_ro/trn_rl_repo")):
    if _os.path.isdir(_p) and _p not in _sys.path:
        _sys.path.append(_p)

"""Builder for the sliding-window attention kernel (NaiveHybridAttention).

Per-core program (SPMD, head-sharded):
  inputs (per core): xT (B,D,S), wqT/wkT/wvT (D,E), woT (E,D),
                     cos/sin RoPE tables (HD,S), pair window masks (128,1024),
                     ones (128,128)
  output: part (B,S,D) = this core's heads' contribution to the final
          out-projection; host sums the 8 partials.

Pipeline per batch:
  A) QKV: qT,kT = W^T-stationary matmuls -> [e, S]; PSUM banks are released
     by a single ACT copy, RoPE runs on DVE from the SBUF copy; v =
     x-stationary -> [s, e].
  B) Attention per head, TRANSPOSED-scores dataflow: for each 256-query
     block, scoresT [k,q] come from kT-tile-stationary matmuls (k on
     partitions) so exp'd probs feed AV directly with NO PE transposes.
     Two adjacent k-tiles share one PSUM bank -> one [128,512] exp per
     pair; the (256,128)-offset pair is fully inside the window (no mask).
     Row-sums via a ones-column matmul; 1/rowsum is computed AFTER an
     outer-product broadcast (full-width DVE reciprocal) and folded into
     the AV-psum evacuation multiply.
  C) Out-proj: attnT-stationary -> psum [s, o] -> DMA to part on the SWDGE
     queue (keeps the HWDGE queue free for the next batch's x stream).

All matmuls run as float32r (full fp32 storage; 1 cycle/row at N>=256).
PSUM: 8 tagged bank slots: S0-S3 (score pairs / qkv q,k / outproj),
AV0-AV2 (AV rotation / qkv v), RS (rowsum+broadcast / qkv v).
"""

import os

import numpy as np
import concourse.bass as bass
from concourse import mybir

F32 = mybir.dt.float32
F32R = mybir.dt.float32r
BF16 = mybir.dt.bfloat16
ROPE_BASE = 10000.0
WINDOW = 512


def r32(ap):
    return ap.bitcast(F32R)


def host_tables(S, HD=128):
    """cos/sin tables in transposed layout [HD, S]. The sin table is
    PARTITION-SWAPPED and sign-folded (rows 0:64 = +sin, rows 64:128 = -sin)
    so each RoPE rot-multiply reads both SBUF inputs from the SAME partition
    range: rot[64:128] = q[0:64]*sin2[0:64], rot[0:64] = q[64:128]*sin2[64:128].
    Unscaled — the softmax 1/sqrt(HD) is applied via the Exp activation's
    scale parameter."""
    inv_freq = 1.0 / (ROPE_BASE ** (np.arange(0, HD, 2, dtype=np.float64) / HD))
    fr = np.arange(S, dtype=np.float64)[None, :] * inv_freq[:, None]  # [HD/2, S]
    cos = np.cos(fr)
    sin = np.sin(fr)
    cos_t = np.concatenate([cos, cos], 0).astype(np.float32)
    sin_sw = np.concatenate([sin, -sin], 0).astype(np.float32)
    return cos_t, sin_sw


def host_masks():
    """Multiplicative (1.0/0.0) sliding-window pair masks in the transposed
    [k, q] orientation. A score tile with offset d0 = q0 - ktile_start is
    valid where 0 <= d0 + qi - ki < WINDOW. Pattern A = tiles (d0=512|384),
    pattern B = tiles (d0=0|-128); the (256|128) pair is fully valid."""

    def m(d0):
        ki = np.arange(128)[:, None]
        qi = np.arange(256)[None, :]
        return ((d0 + qi - ki >= 0) & (d0 + qi - ki < WINDOW)).astype(np.float32)

    pa = np.concatenate([m(512), m(384)], axis=1)   # [128, 512]
    pb = np.concatenate([m(0), m(-128)], axis=1)    # [128, 512]
    return np.concatenate([pa, pb], axis=1)          # [128, 1024]


def partial_ref_np(x, wq_r, wk_r, wv_r, wo_t):
    """NumPy mirror of the per-core computation (fp32).
    x: (B,S,D); wq_r/wk_r/wv_r: (E,D) row-slices of w_qkv; wo_t: (E,D) =
    w_out[:, e_slice].T. Returns (B,S,D) partial."""
    B, S, D = x.shape
    E = wq_r.shape[0]
    HC = E // 128
    q = np.einsum("bsd,ed->bse", x, wq_r).reshape(B, S, HC, 128)
    k = np.einsum("bsd,ed->bse", x, wk_r).reshape(B, S, HC, 128)
    v = np.einsum("bsd,ed->bse", x, wv_r).reshape(B, S, HC, 128)
    inv_freq = 1.0 / (ROPE_BASE ** (np.arange(0, 128, 2, dtype=np.float64) / 128))
    fr = np.arange(S, dtype=np.float64)[:, None] * inv_freq[None, :]
    emb = np.concatenate([fr, fr], -1)
    cos = np.cos(emb).astype(np.float32)[None, :, None, :]
    sin = np.sin(emb).astype(np.float32)[None, :, None, :]

    def rot(t):
        t1, t2 = t[..., :64], t[..., 64:]
        return np.concatenate([-t2, t1], -1)

    q = q * cos + rot(q) * sin
    k = k * cos + rot(k) * sin
    scale = 1.0 / np.sqrt(128.0)
    i = np.arange(S)[:, None]
    j = np.arange(S)[None, :]
    valid = (i - j >= 0) & (i - j < WINDOW)
    out = np.zeros((B, S, E), np.float32)
    for b in range(B):
        for h in range(HC):
            s = (q[b, :, h] @ k[b, :, h].T) * scale
            s = np.where(valid, s, -np.inf)
            s = s - s.max(-1, keepdims=True)
            p = np.exp(s)
            p /= p.sum(-1, keepdims=True)
            out[b, :, h * 128 : (h + 1) * 128] = p @ v[b, :, h]
    return np.einsum("bse,ed->bsd", out, wo_t).astype(np.float32)


def declare_io(nc, B, S, D, E):
    dt = F32
    t = {}
    t["xt"] = nc.dram_tensor("xt", [B, D, S], dt, kind="ExternalInput").ap()
    for n in ("wqt", "wkt", "wvt"):
        t[n] = nc.dram_tensor(n, [D, E], dt, kind="ExternalInput").ap()
    t["wot"] = nc.dram_tensor("wot", [E, D], dt, kind="ExternalInput").ap()
    for n in ("cost", "sint"):
        t[n] = nc.dram_tensor(n, [128, S], dt, kind="ExternalInput").ap()
    t["maskp"] = nc.dram_tensor("maskp", [128, 1024], dt, kind="ExternalInput").ap()
    t["ones"] = nc.dram_tensor("ones", [128, 128], dt, kind="ExternalInput").ap()
    # partial written bf16: halves the dominant output-DMA stream; the 8
    # per-core partials are summed in float64 on the host
    t["part"] = nc.dram_tensor("part", [B, S, D], BF16, kind="ExternalOutput").ap()
    return t


def build_program(ctx, nc, tc, io, B, S, D, HC, reps=1):
    """Emit the per-core program. HC = heads on this core; E = HC*128.
    reps > 1 wraps the body in a hardware loop repeating the identical
    computation (for timing measurements); output is unchanged."""
    E = HC * 128
    KT = D // 128  # contraction tiles for qkv

    const = ctx.enter_context(tc.tile_pool(name="const", bufs=1))
    work = ctx.enter_context(tc.tile_pool(name="work", bufs=1))
    xsp = ctx.enter_context(tc.tile_pool(name="xs", bufs=5))
    tmp = ctx.enter_context(tc.tile_pool(name="tmp", bufs=2))
    pp = ctx.enter_context(tc.tile_pool(name="pp", bufs=2))
    rbp = ctx.enter_context(tc.tile_pool(name="rb", bufs=2))
    outp = ctx.enter_context(tc.tile_pool(name="outp", bufs=2))
    ps = ctx.enter_context(tc.tile_pool(name="ps", bufs=1, space="PSUM"))

    # ---- constants ----
    # q/k/v weights: one DMA per 128-row k-tile so the first matmuls only
    # depend on the slices they read (kills the startup stall). Other consts
    # go on the gpsimd (SWDGE) queue to stay off the HWDGE queue that
    # streams x.
    wq_sb = const.tile([128, KT, E], F32R)
    wk_sb = const.tile([128, KT, E], F32R)
    wv_sb = const.tile([128, KT, E], F32R)
    for kt in range(KT):
        rows = bass.ts(kt, 128)
        nc.gpsimd.dma_start(wq_sb[:, kt, :], r32(io["wqt"][rows, :]))
        nc.gpsimd.dma_start(wk_sb[:, kt, :], r32(io["wkt"][rows, :]))
        nc.gpsimd.dma_start(wv_sb[:, kt, :], r32(io["wvt"][rows, :]))
    wo_sb = const.tile([128, HC, D], F32R)
    nc.gpsimd.dma_start(wo_sb[:], r32(io["wot"].rearrange("(et p) o -> p et o", p=128)))
    cost = const.tile([128, S], F32)
    nc.gpsimd.dma_start(cost[:], io["cost"][:])
    sint = const.tile([128, S], F32)
    nc.gpsimd.dma_start(sint[:], io["sint"][:])
    maskp = const.tile([128, 1024], F32)
    nc.gpsimd.dma_start(maskp[:], io["maskp"][:])
    ones_sb = const.tile([128, 128], F32R)
    nc.gpsimd.dma_start(ones_sb[:], r32(io["ones"][:]))

    def rope(dst, src_ps, cos_t, sin_t, cols, w):
        """dst[:, cols] = src_ps*cos + swap_halves(src_ps)*sin (RoPE).
        One ACT copy releases the PSUM bank; the DVE chain then reads the
        SBUF copy, so the next chunk's matmuls are not gated on DVE."""
        sb = tmp.tile([128, 512], F32, tag="pcp")
        nc.scalar.copy(sb[:, :w], src_ps[:, :w])
        rot = tmp.tile([128, 512], F32, tag="rot")
        nc.vector.tensor_mul(rot[0:64, :w], sb[64:128, :w], sin_t[64:128, cols])
        nc.vector.tensor_mul(rot[64:128, :w], sb[0:64, :w], sin_t[0:64, cols])
        cv = tmp.tile([128, 512], F32, tag="cosv")
        nc.vector.tensor_mul(cv[:, :w], sb[:, :w], cos_t[:, cols])
        nc.vector.tensor_add(dst, cv[:, :w], rot[:, :w])

    def body():
        _emit_body(nc, tc, io, B, S, D, HC, locals_=dict(
            const=const, work=work, xsp=xsp, tmp=tmp, pp=pp,
            rbp=rbp, outp=outp, ps=ps,
            wq_sb=wq_sb, wk_sb=wk_sb, wv_sb=wv_sb, wo_sb=wo_sb,
            cost=cost, sint=sint, maskp=maskp, ones_sb=ones_sb, rope=rope,
        ))

    if reps > 1:
        with tc.For_i(0, reps, 1):
            body()
    else:
        body()


def _emit_body(nc, tc, io, B, S, D, HC, locals_):
    E = HC * 128
    KT = D // 128
    SC = S // 512
    QB = S // 256
    ST = S // 128
    OCW = min(512, D)
    OC = D // OCW
    work = locals_["work"]; xsp = locals_["xsp"]
    pp = locals_["pp"]; rbp = locals_["rbp"]
    outp = locals_["outp"]; ps = locals_["ps"]
    wq_sb = locals_["wq_sb"]; wk_sb = locals_["wk_sb"]; wv_sb = locals_["wv_sb"]
    wo_sb = locals_["wo_sb"]; cost = locals_["cost"]; sint = locals_["sint"]
    maskp = locals_["maskp"]; ones_sb = locals_["ones_sb"]
    rope = locals_["rope"]

    for b in range(B):
        # ---- A) QKV projection ----
        qT = work.tile([128, HC, S], F32R, tag="qT")
        kT = work.tile([128, HC, S], F32R, tag="kT")
        v_sb = work.tile([128, ST, E], F32R, tag="v")
        for sc in range(SC):
            cols = bass.ts(sc, 512)
            q_ps = [
                ps.tile([128, 512], F32, tag=t, name=f"q_ps{i}")
                for i, t in enumerate(("S0", "S1")[:HC])
            ]
            k_ps = [
                ps.tile([128, 512], F32, tag=t, name=f"k_ps{i}")
                for i, t in enumerate(("S2", "S3")[:HC])
            ]
            v_ps = [
                ps.tile([128, E], F32, tag=t, name=f"v_ps{i}")
                for i, t in enumerate(("AV0", "AV1", "AV2", "RS"))
            ]
            for kt in range(KT):
                xs = xsp.tile([128, 512], F32R)
                nc.sync.dma_start(xs[:], r32(io["xt"][b, bass.ts(kt, 128), cols]))
                f = dict(start=(kt == 0), stop=(kt == KT - 1))
                # v first: its psum slots are evacuated fastest, so the next
                # chunk's accumulation can begin while q/k RoPE evac runs
                for ss in range(4):
                    nc.tensor.matmul(
                        v_ps[ss][:],
                        xs[:, bass.ts(ss, 128)],
                        wv_sb[:, kt, :],
                        **f,
                    )
                for et in range(HC):
                    nc.tensor.matmul(
                        k_ps[et][:], wk_sb[:, kt, bass.ts(et, 128)], xs[:], **f
                    )
                    nc.tensor.matmul(
                        q_ps[et][:], wq_sb[:, kt, bass.ts(et, 128)], xs[:], **f
                    )
            for et in range(HC):
                rope(qT[:, et, cols], q_ps[et], cost, sint, cols, 512)
                rope(kT[:, et, cols], k_ps[et], cost, sint, cols, 512)
            for ss in range(4):
                nc.scalar.copy(v_sb[:, sc * 4 + ss, :], v_ps[ss][:])

        # ---- B) attention, per head (transposed-scores dataflow) ----
        # Per 256-query block: up to 3 PAIRS of scoresT [k,q] tiles, each
        # pair filling one [128,512] PSUM bank (2 matmuls), one exp per
        # pair on ACT, pair mask on DVE (middle pair needs none),
        # ones-matmul row-sums + AV on PE, normalization via broadcast +
        # full-width reciprocal folded into the AV evacuation multiply.
        attnT = work.tile([128, HC, S], F32R, tag="attnT")
        exp_scale = float(1.0 / np.sqrt(128.0))
        state = dict(gidx=0, blk=0)

        def emit_front(h, qb):
            """Score matmuls + exp + mask for block (h, qb)."""
            q0 = qb * 256
            kstart = max(0, q0 - WINDOW)
            nkt = (q0 + 256 - kstart) // 128
            qcols = bass.ds(q0, 256)
            ptiles = []
            for pr in range(nkt // 2):
                kt0 = 2 * pr
                d0 = q0 - kstart - 128 * kt0
                sp = ps.tile(
                    [128, 512], F32, tag=f"S{state['gidx'] % 4}", name="sp"
                )
                state["gidx"] += 1
                for t in range(2):
                    nc.tensor.matmul(
                        sp[:, bass.ts(t, 256)],
                        kT[:, h, bass.ds(kstart + 128 * (kt0 + t), 128)],
                        qT[:, h, qcols],
                    )
                p_sb = pp.tile([128, 512], F32R, tag=f"p{pr}", name=f"p{pr}")
                nc.scalar.activation(
                    p_sb[:], sp[:], mybir.ActivationFunctionType.Exp,
                    scale=exp_scale,
                )
                if d0 == 512:
                    nc.vector.tensor_mul(p_sb[:], p_sb[:], maskp[:, 0:512])
                elif d0 == 0:
                    nc.vector.tensor_mul(p_sb[:], p_sb[:], maskp[:, 512:1024])
                # d0 == 256: pair fully inside the window, no mask
                ptiles.append(p_sb)
            return dict(h=h, q0=q0, kstart=kstart, nkt=nkt, p=ptiles)

        def emit_tail(d):
            """Row-sums, AV, broadcast-normalize for a previously-issued
            block."""
            h, q0, kstart, nkt = d["h"], d["q0"], d["kstart"], d["nkt"]
            avx = state["blk"] % 3
            state["blk"] += 1
            rs = ps.tile([1, 256], F32, tag="RS", name="rs")
            for kt in range(nkt):
                nc.tensor.matmul(
                    rs[:], ones_sb[:, 0:1],
                    d["p"][kt // 2][:, bass.ts(kt % 2, 256)],
                    start=(kt == 0), stop=(kt == nkt - 1),
                )
            rs_sb = rbp.tile([1, 256], F32R, tag="rss")
            nc.scalar.copy(rs_sb[:], rs[:])
            av = ps.tile([128, 256], F32, tag=f"AV{avx}", name="av")
            for kt in range(nkt):
                nc.tensor.matmul(
                    av[:],
                    v_sb[:, kstart // 128 + kt, bass.ts(h, 128)],
                    d["p"][kt // 2][:, bass.ts(kt % 2, 256)],
                    start=(kt == 0), stop=(kt == nkt - 1),
                )
            rb_ps = ps.tile([128, 256], F32, tag="RS", name="rb")
            nc.tensor.matmul(rb_ps[:], ones_sb[0:1, :], rs_sb[:])
            rb_sb = rbp.tile([128, 256], F32, tag="rbs")
            nc.vector.reciprocal(rb_sb[:], rb_ps[:])
            nc.vector.tensor_mul(attnT[:, h, bass.ds(q0, 256)], av[:], rb_sb[:])

        # one-stage software pipeline: scores+exp of block i are issued
        # before the rowsum/AV/normalize of block i-1, so PE streams through
        # the ACT/DVE softmax chain
        prev = None
        for h in range(HC):
            for qb in range(QB):
                d = emit_front(h, qb)
                if prev is not None:
                    emit_tail(prev)
                prev = d
        emit_tail(prev)

        # ---- C) out-projection (partial over this core's E dims) ----
        # evac copies alternate DVE/ACT; one batched 1 MB output DMA per
        # 128-row stripe, on the SWDGE queue so the HWDGE queue stays free
        # for the next batch's x stream
        for st in range(ST):
            osb = outp.tile([128, D], BF16, tag="osb")
            for oc in range(OC):
                o_ps = ps.tile(
                    [128, OCW], F32, tag=("S0" if oc % 2 == 0 else "S1"), name="o_ps"
                )
                for et in range(HC):
                    nc.tensor.matmul(
                        o_ps[:],
                        attnT[:, et, bass.ts(st, 128)],
                        wo_sb[:, et, bass.ts(oc, OCW)],
                        start=(et == 0),
                        stop=(et == HC - 1),
                    )
                dst = osb[:, bass.ts(oc, OCW)]
                if oc % 2 == 0:
                    nc.vector.tensor_copy(dst, o_ps[:])
                else:
                    nc.scalar.copy(dst, o_ps[:])
            nc.gpsimd.dma_start(io["part"][b, bass.ts(st, 128), :], osb[:])


# ======================================================================
# 8-core SPMD wrapper
# ======================================================================
from contextlib import ExitStack as _ExitStack

N_CORES = 8
B_FULL, S_FULL, D_FULL, H_FULL, HD_FULL = 2, 2048, 2048, 16, 128
HC_FULL = H_FULL // N_CORES  # 2 heads per core

_nc_cache = {}


def get_compiled(reps=1):
    """Build + bacc-compile the per-core Bass program (cached per reps)."""
    if reps not in _nc_cache:
        import concourse.bacc as bacc
        from concourse import tile

        nc = bacc.Bacc(
            "TRN2", target_bir_lowering=False, debug=False, num_devices=N_CORES
        )
        io = declare_io(nc, B_FULL, S_FULL, D_FULL, HC_FULL * 128)
        with tile.TileContext(nc) as tc:
            with _ExitStack() as ctx:
                build_program(
                    ctx, nc, tc, io, B_FULL, S_FULL, D_FULL, HC_FULL, reps=reps
                )
        nc.compile()
        _nc_cache[reps] = nc
    return _nc_cache[reps]


def make_in_maps(x, w_qkv, w_out):
    """Host-side sharding: per-core input dicts (head-sharded)."""
    x = np.ascontiguousarray(np.asarray(x, dtype=np.float32))
    w_qkv = np.ascontiguousarray(np.asarray(w_qkv, dtype=np.float32))
    w_out = np.ascontiguousarray(np.asarray(w_out, dtype=np.float32))
    D = D_FULL
    xt = np.ascontiguousarray(x.transpose(0, 2, 1))
    cos_t, sin_t = host_tables(S_FULL)
    maskp = host_masks()
    ones = np.ones((128, 128), np.float32)
    in_maps = []
    for c in range(N_CORES):
        e0, e1 = c * HC_FULL * 128, (c + 1) * HC_FULL * 128
        in_maps.append(
            dict(
                xt=xt,
                wqt=np.ascontiguousarray(w_qkv[e0:e1].T),
                wkt=np.ascontiguousarray(w_qkv[D + e0 : D + e1].T),
                wvt=np.ascontiguousarray(w_qkv[2 * D + e0 : 2 * D + e1].T),
                wot=np.ascontiguousarray(w_out[:, e0:e1].T),
                cost=cos_t,
                sint=sin_t,
                maskp=maskp,
                ones=ones,
            )
        )
    return in_maps


def combine(parts):
    """Sum the 8 per-core out-projection partials."""
    acc = np.zeros((B_FULL, S_FULL, D_FULL), np.float64)
    for p in parts:
        acc += p
    return acc.astype(np.float32)


def kernel(x, w_qkv, w_out):
    from concourse import bass_utils

    nc = get_compiled(reps=1)
    in_maps = make_in_maps(x, w_qkv, w_out)
    res = bass_utils.run_bass_kernel_spmd(
        nc, in_maps, core_ids=list(range(N_CORES))
    )
    return combine([res.results[c]["part"] for c in range(N_CORES)])


# revision 29
# speedup vs baseline: 1.1887x; 1.1032x over previous
import sys as _sys
import os as _os

for _p in ("/opt/trn_rl_repo", _os.path.expanduser("~/.axon_site/_ro/trn_rl_repo")):
    if _os.path.isdir(_p) and _p not in _sys.path:
        _sys.path.append(_p)

"""Builder for the sliding-window attention kernel (NaiveHybridAttention).

Per-core program (SPMD, head-sharded):
  inputs (per core): xT (B,D,S), wqT/wkT/wvT (D,E), woT (E,D),
                     cos/sin RoPE tables (HD,S), pair window masks (128,1024),
                     ones (128,128)
  output: part (B,S,D) = this core's heads' contribution to the final
          out-projection; host sums the 8 partials.

Pipeline per batch:
  A) QKV: qT,kT = W^T-stationary matmuls -> [e, S]; PSUM banks are released
     by a single ACT copy, RoPE runs on DVE from the SBUF copy; v =
     x-stationary -> [s, e].
  B) Attention per head, TRANSPOSED-scores dataflow: for each 256-query
     block, scoresT [k,q] come from kT-tile-stationary matmuls (k on
     partitions) so exp'd probs feed AV directly with NO PE transposes.
     Two adjacent k-tiles share one PSUM bank -> one [128,512] exp per
     pair; the (256,128)-offset pair is fully inside the window (no mask).
     Row-sums via a ones-column matmul; 1/rowsum is computed AFTER an
     outer-product broadcast (full-width DVE reciprocal) and folded into
     the AV-psum evacuation multiply.
  C) Out-proj: attnT-stationary -> psum [s, o] -> DMA to part on the SWDGE
     queue (keeps the HWDGE queue free for the next batch's x stream).

All matmuls run as float32r (full fp32 storage; 1 cycle/row at N>=256).
PSUM: 8 tagged bank slots: S0-S3 (score pairs / qkv q,k / outproj),
AV0-AV2 (AV rotation / qkv v), RS (rowsum+broadcast / qkv v).
"""

import os

import numpy as np
import concourse.bass as bass
from concourse import mybir

F32 = mybir.dt.float32
F32R = mybir.dt.float32r
BF16 = mybir.dt.bfloat16
ROPE_BASE = 10000.0
WINDOW = 512


def r32(ap):
    return ap.bitcast(F32R)


def host_tables(S, HD=128):
    """cos/sin tables in transposed layout [HD, S]. The sin table is
    PARTITION-SWAPPED and sign-folded (rows 0:64 = +sin, rows 64:128 = -sin)
    so each RoPE rot-multiply reads both SBUF inputs from the SAME partition
    range: rot[64:128] = q[0:64]*sin2[0:64], rot[0:64] = q[64:128]*sin2[64:128].
    Unscaled — the softmax 1/sqrt(HD) is applied via the Exp activation's
    scale parameter."""
    inv_freq = 1.0 / (ROPE_BASE ** (np.arange(0, HD, 2, dtype=np.float64) / HD))
    fr = np.arange(S, dtype=np.float64)[None, :] * inv_freq[:, None]  # [HD/2, S]
    cos = np.cos(fr)
    sin = np.sin(fr)
    cos_t = np.concatenate([cos, cos], 0).astype(np.float32)
    sin_sw = np.concatenate([sin, -sin], 0).astype(np.float32)
    return cos_t, sin_sw


def host_masks():
    """Multiplicative (1.0/0.0) sliding-window pair masks in the transposed
    [k, q] orientation. A score tile with offset d0 = q0 - ktile_start is
    valid where 0 <= d0 + qi - ki < WINDOW. Pattern A = tiles (d0=512|384),
    pattern B = tiles (d0=0|-128); the (256|128) pair is fully valid."""

    def m(d0):
        ki = np.arange(128)[:, None]
        qi = np.arange(256)[None, :]
        return ((d0 + qi - ki >= 0) & (d0 + qi - ki < WINDOW)).astype(np.float32)

    pa = np.concatenate([m(512), m(384)], axis=1)   # [128, 512]
    pb = np.concatenate([m(0), m(-128)], axis=1)    # [128, 512]
    return np.concatenate([pa, pb], axis=1)          # [128, 1024]


def partial_ref_np(x, wq_r, wk_r, wv_r, wo_t):
    """NumPy mirror of the per-core computation (fp32).
    x: (B,S,D); wq_r/wk_r/wv_r: (E,D) row-slices of w_qkv; wo_t: (E,D) =
    w_out[:, e_slice].T. Returns (B,S,D) partial."""
    B, S, D = x.shape
    E = wq_r.shape[0]
    HC = E // 128
    q = np.einsum("bsd,ed->bse", x, wq_r).reshape(B, S, HC, 128)
    k = np.einsum("bsd,ed->bse", x, wk_r).reshape(B, S, HC, 128)
    v = np.einsum("bsd,ed->bse", x, wv_r).reshape(B, S, HC, 128)
    inv_freq = 1.0 / (ROPE_BASE ** (np.arange(0, 128, 2, dtype=np.float64) / 128))
    fr = np.arange(S, dtype=np.float64)[:, None] * inv_freq[None, :]
    emb = np.concatenate([fr, fr], -1)
    cos = np.cos(emb).astype(np.float32)[None, :, None, :]
    sin = np.sin(emb).astype(np.float32)[None, :, None, :]

    def rot(t):
        t1, t2 = t[..., :64], t[..., 64:]
        return np.concatenate([-t2, t1], -1)

    q = q * cos + rot(q) * sin
    k = k * cos + rot(k) * sin
    scale = 1.0 / np.sqrt(128.0)
    i = np.arange(S)[:, None]
    j = np.arange(S)[None, :]
    valid = (i - j >= 0) & (i - j < WINDOW)
    out = np.zeros((B, S, E), np.float32)
    for b in range(B):
        for h in range(HC):
            s = (q[b, :, h] @ k[b, :, h].T) * scale
            s = np.where(valid, s, -np.inf)
            s = s - s.max(-1, keepdims=True)
            p = np.exp(s)
            p /= p.sum(-1, keepdims=True)
            out[b, :, h * 128 : (h + 1) * 128] = p @ v[b, :, h]
    return np.einsum("bse,ed->bsd", out, wo_t).astype(np.float32)


def declare_io(nc, B, S, D, E):
    dt = F32
    t = {}
    # x and the qkv weights stream in bf16: halves the dominant input-DMA
    # stream; accumulation stays fp32 in PSUM
    t["xt"] = nc.dram_tensor("xt", [B, D, S], BF16, kind="ExternalInput").ap()
    for n in ("wqt", "wkt", "wvt"):
        t[n] = nc.dram_tensor(n, [D, E], BF16, kind="ExternalInput").ap()
    t["wot"] = nc.dram_tensor("wot", [E, D], dt, kind="ExternalInput").ap()
    for n in ("cost", "sint"):
        t[n] = nc.dram_tensor(n, [128, S], dt, kind="ExternalInput").ap()
    t["maskp"] = nc.dram_tensor("maskp", [128, 1024], dt, kind="ExternalInput").ap()
    t["ones"] = nc.dram_tensor("ones", [128, 128], dt, kind="ExternalInput").ap()
    # partial written bf16: halves the dominant output-DMA stream; the 8
    # per-core partials are summed in float64 on the host
    t["part"] = nc.dram_tensor("part", [B, S, D], BF16, kind="ExternalOutput").ap()
    return t


def build_program(ctx, nc, tc, io, B, S, D, HC, reps=1):
    """Emit the per-core program. HC = heads on this core; E = HC*128.
    reps > 1 wraps the body in a hardware loop repeating the identical
    computation (for timing measurements); output is unchanged."""
    E = HC * 128
    KT = D // 128  # contraction tiles for qkv

    const = ctx.enter_context(tc.tile_pool(name="const", bufs=1))
    work = ctx.enter_context(tc.tile_pool(name="work", bufs=1))
    xsp = ctx.enter_context(tc.tile_pool(name="xs", bufs=5))
    tmp = ctx.enter_context(tc.tile_pool(name="tmp", bufs=2))
    pp = ctx.enter_context(tc.tile_pool(name="pp", bufs=2))
    rbp = ctx.enter_context(tc.tile_pool(name="rb", bufs=2))
    outp = ctx.enter_context(tc.tile_pool(name="outp", bufs=3))
    ps = ctx.enter_context(tc.tile_pool(name="ps", bufs=1, space="PSUM"))

    # ---- constants ----
    # q/k/v weights: one DMA per 128-row k-tile so the first matmuls only
    # depend on the slices they read (kills the startup stall). Other consts
    # go on the gpsimd (SWDGE) queue to stay off the HWDGE queue that
    # streams x.
    wq_sb = const.tile([128, KT, E], BF16)
    wk_sb = const.tile([128, KT, E], BF16)
    wv_sb = const.tile([128, KT, E], BF16)
    for kt in range(KT):
        rows = bass.ts(kt, 128)
        nc.gpsimd.dma_start(wq_sb[:, kt, :], io["wqt"][rows, :])
        nc.gpsimd.dma_start(wk_sb[:, kt, :], io["wkt"][rows, :])
        nc.gpsimd.dma_start(wv_sb[:, kt, :], io["wvt"][rows, :])
    wo_sb = const.tile([128, HC, D], F32R)
    nc.gpsimd.dma_start(wo_sb[:], r32(io["wot"].rearrange("(et p) o -> p et o", p=128)))
    cost = const.tile([128, S], F32)
    nc.gpsimd.dma_start(cost[:], io["cost"][:])
    sint = const.tile([128, S], F32)
    nc.gpsimd.dma_start(sint[:], io["sint"][:])
    maskp = const.tile([128, 1024], F32)
    nc.gpsimd.dma_start(maskp[:], io["maskp"][:])
    ones_sb = const.tile([128, 128], F32R)
    nc.gpsimd.dma_start(ones_sb[:], r32(io["ones"][:]))

    def rope_evac(src_ps, w, slot):
        """Release a q/k PSUM bank with a single DVE copy (first in the DVE
        queue for the chunk, so the bank frees before the next chunk's
        matmuls need it). Returns the SBUF copy."""
        sb = tmp.tile([128, 512], F32, tag=f"pcp{slot}")
        nc.vector.tensor_copy(sb[:, :w], src_ps[:, :w])
        return sb

    def rope(dst, sb, cos_t, sin_t, cols, w):
        """dst[:, cols] = sb*cos + swap_halves(sb)*sin (RoPE), from the SBUF
        copy made by rope_evac. The cos product lands in dst; rot is added
        in place."""
        rot = tmp.tile([128, 512], F32, tag="rot")
        nc.vector.tensor_mul(rot[0:64, :w], sb[64:128, :w], sin_t[64:128, cols])
        nc.vector.tensor_mul(rot[64:128, :w], sb[0:64, :w], sin_t[0:64, cols])
        nc.vector.tensor_mul(dst, sb[:, :w], cos_t[:, cols])
        nc.vector.tensor_add(dst, dst, rot[:, :w])

    def body():
        _emit_body(nc, tc, io, B, S, D, HC, locals_=dict(
            const=const, work=work, xsp=xsp, tmp=tmp, pp=pp,
            rbp=rbp, outp=outp, ps=ps,
            wq_sb=wq_sb, wk_sb=wk_sb, wv_sb=wv_sb, wo_sb=wo_sb,
            cost=cost, sint=sint, maskp=maskp, ones_sb=ones_sb, rope=rope,
            rope_evac=rope_evac,
        ))

    if reps > 1:
        with tc.For_i(0, reps, 1):
            body()
    else:
        body()


def _emit_body(nc, tc, io, B, S, D, HC, locals_):
    E = HC * 128
    KT = D // 128
    SC = S // 512
    QB = S // 256
    ST = S // 128
    OCW = min(512, D)
    OC = D // OCW
    work = locals_["work"]; xsp = locals_["xsp"]
    pp = locals_["pp"]; rbp = locals_["rbp"]
    outp = locals_["outp"]; ps = locals_["ps"]
    wq_sb = locals_["wq_sb"]; wk_sb = locals_["wk_sb"]; wv_sb = locals_["wv_sb"]
    wo_sb = locals_["wo_sb"]; cost = locals_["cost"]; sint = locals_["sint"]
    maskp = locals_["maskp"]; ones_sb = locals_["ones_sb"]
    rope = locals_["rope"]
    rope_evac = locals_["rope_evac"]

    for b in range(B):
        # ---- A) QKV projection ----
        qT = work.tile([128, HC, S], F32R, tag="qT")
        kT = work.tile([128, HC, S], F32R, tag="kT")
        v_sb = work.tile([128, ST, E], F32R, tag="v")
        for sc in range(SC):
            cols = bass.ts(sc, 512)
            q_ps = [
                ps.tile([128, 512], F32, tag=t, name=f"q_ps{i}")
                for i, t in enumerate(("S0", "S1")[:HC])
            ]
            k_ps = [
                ps.tile([128, 512], F32, tag=t, name=f"k_ps{i}")
                for i, t in enumerate(("S2", "S3")[:HC])
            ]
            v_ps = [
                ps.tile([128, E], F32, tag=t, name=f"v_ps{i}")
                for i, t in enumerate(("AV0", "AV1", "AV2", "RS"))
            ]
            for kt in range(KT):
                xs = xsp.tile([128, 512], BF16)
                nc.sync.dma_start(xs[:], io["xt"][b, bass.ts(kt, 128), cols])
                f = dict(start=(kt == 0), stop=(kt == KT - 1))
                # v first: its psum slots are evacuated fastest, so the next
                # chunk's accumulation can begin while q/k RoPE evac runs
                for ss in range(4):
                    nc.tensor.matmul(
                        v_ps[ss][:],
                        xs[:, bass.ts(ss, 128)],
                        wv_sb[:, kt, :],
                        **f,
                    )
                for et in range(HC):
                    nc.tensor.matmul(
                        k_ps[et][:], wk_sb[:, kt, bass.ts(et, 128)], xs[:], **f
                    )
                    nc.tensor.matmul(
                        q_ps[et][:], wq_sb[:, kt, bass.ts(et, 128)], xs[:], **f
                    )
            # v-bank releases on ACT (first in its queue), q/k-bank releases
            # on DVE, both in next-chunk consumption order (v, then k,q per
            # head); rope math follows from the SBUF copies
            for ss in range(4):
                nc.scalar.copy(v_sb[:, sc * 4 + ss, :], v_ps[ss][:])
            sbs = []
            for et in range(HC):
                sbk = rope_evac(k_ps[et], 512, 2 * et)
                sbq = rope_evac(q_ps[et], 512, 2 * et + 1)
                sbs.append((sbk, sbq))
            for et in range(HC):
                sbk, sbq = sbs[et]
                rope(kT[:, et, cols], sbk, cost, sint, cols, 512)
                rope(qT[:, et, cols], sbq, cost, sint, cols, 512)

        # ---- B) attention, per head (transposed-scores dataflow) ----
        # Per 256-query block: up to 3 PAIRS of scoresT [k,q] tiles, each
        # pair filling one [128,512] PSUM bank (2 matmuls), one exp per
        # pair on ACT, pair mask on DVE (middle pair needs none),
        # ones-matmul row-sums + AV on PE, normalization via broadcast +
        # full-width reciprocal folded into the AV evacuation multiply.
        attnT = work.tile([128, HC, S], F32R, tag="attnT")
        exp_scale = float(1.0 / np.sqrt(128.0))
        state = dict(gidx=0, blk=0)

        def emit_front(h, qb):
            """Score matmuls + exp + mask for block (h, qb)."""
            q0 = qb * 256
            kstart = max(0, q0 - WINDOW)
            nkt = (q0 + 256 - kstart) // 128
            qcols = bass.ds(q0, 256)
            ptiles = []
            for pr in range(nkt // 2):
                kt0 = 2 * pr
                d0 = q0 - kstart - 128 * kt0
                sp = ps.tile(
                    [128, 512], F32, tag=f"S{state['gidx'] % 4}", name="sp"
                )
                state["gidx"] += 1
                for t in range(2):
                    nc.tensor.matmul(
                        sp[:, bass.ts(t, 256)],
                        kT[:, h, bass.ds(kstart + 128 * (kt0 + t), 128)],
                        qT[:, h, qcols],
                    )
                p_sb = pp.tile([128, 512], F32R, tag=f"p{pr}", name=f"p{pr}")
                nc.scalar.activation(
                    p_sb[:], sp[:], mybir.ActivationFunctionType.Exp,
                    scale=exp_scale,
                )
                if d0 == 512:
                    nc.vector.tensor_mul(p_sb[:], p_sb[:], maskp[:, 0:512])
                elif d0 == 0:
                    nc.vector.tensor_mul(p_sb[:], p_sb[:], maskp[:, 512:1024])
                # d0 == 256: pair fully inside the window, no mask
                ptiles.append(p_sb)
            return dict(h=h, q0=q0, kstart=kstart, nkt=nkt, p=ptiles)

        def emit_tail(d):
            """Row-sums, AV, broadcast-normalize for a previously-issued
            block."""
            h, q0, kstart, nkt = d["h"], d["q0"], d["kstart"], d["nkt"]
            avx = state["blk"] % 3
            state["blk"] += 1
            rs = ps.tile([1, 256], F32, tag="RS", name="rs")
            for kt in range(nkt):
                nc.tensor.matmul(
                    rs[:], ones_sb[:, 0:1],
                    d["p"][kt // 2][:, bass.ts(kt % 2, 256)],
                    start=(kt == 0), stop=(kt == nkt - 1),
                )
            rs_sb = rbp.tile([1, 256], F32R, tag="rss")
            nc.vector.tensor_copy(rs_sb[:], rs[:])
            av = ps.tile([128, 256], F32, tag=f"AV{avx}", name="av")
            for kt in range(nkt):
                nc.tensor.matmul(
                    av[:],
                    v_sb[:, kstart // 128 + kt, bass.ts(h, 128)],
                    d["p"][kt // 2][:, bass.ts(kt % 2, 256)],
                    start=(kt == 0), stop=(kt == nkt - 1),
                )
            rb_ps = ps.tile([128, 256], F32, tag="RS", name="rb")
            nc.tensor.matmul(rb_ps[:], ones_sb[0:1, :], rs_sb[:])
            rb_sb = rbp.tile([128, 256], F32, tag="rbs")
            nc.vector.reciprocal(rb_sb[:], rb_ps[:])
            nc.vector.tensor_mul(attnT[:, h, bass.ds(q0, 256)], av[:], rb_sb[:])

        # two-stage software pipeline: scores+exp run two blocks ahead of
        # rowsum/AV/normalize, so PE streams through the ACT/DVE softmax
        # chain even across the phase-entry DVE queue drain
        from collections import deque

        pend = deque()
        for h in range(HC):
            for qb in range(QB):
                pend.append(emit_front(h, qb))
                if len(pend) > 2:
                    emit_tail(pend.popleft())
        while pend:
            emit_tail(pend.popleft())

        # ---- C) out-projection (partial over this core's E dims) ----
        # evac copies alternate DVE/ACT; one batched 1 MB output DMA per
        # 128-row stripe, on the SWDGE queue so the HWDGE queue stays free
        # for the next batch's x stream
        for st in range(ST):
            osb = outp.tile([128, D], BF16, tag="osb")
            for oc in range(OC):
                o_ps = ps.tile(
                    [128, OCW], F32, tag=f"S{oc % 4}", name="o_ps"
                )
                for et in range(HC):
                    nc.tensor.matmul(
                        o_ps[:],
                        attnT[:, et, bass.ts(st, 128)],
                        wo_sb[:, et, bass.ts(oc, OCW)],
                        start=(et == 0),
                        stop=(et == HC - 1),
                    )
                dst = osb[:, bass.ts(oc, OCW)]
                if oc % 2 == 0:
                    nc.vector.tensor_copy(dst, o_ps[:])
                else:
                    nc.scalar.copy(dst, o_ps[:])
            nc.gpsimd.dma_start(io["part"][b, bass.ts(st, 128), :], osb[:])


# ======================================================================
# 8-core SPMD wrapper
# ======================================================================
from contextlib import ExitStack as _ExitStack

N_CORES = 8
B_FULL, S_FULL, D_FULL, H_FULL, HD_FULL = 2, 2048, 2048, 16, 128
HC_FULL = H_FULL // N_CORES  # 2 heads per core

_nc_cache = {}


def get_compiled(reps=1):
    """Build + bacc-compile the per-core Bass program (cached per reps)."""
    if reps not in _nc_cache:
        import concourse.bacc as bacc
        from concourse import tile

        nc = bacc.Bacc(
            "TRN2", target_bir_lowering=False, debug=False, num_devices=N_CORES
        )
        io = declare_io(nc, B_FULL, S_FULL, D_FULL, HC_FULL * 128)
        with tile.TileContext(nc) as tc:
            with _ExitStack() as ctx:
                build_program(
                    ctx, nc, tc, io, B_FULL, S_FULL, D_FULL, HC_FULL, reps=reps
                )
        nc.compile()
        _nc_cache[reps] = nc
    return _nc_cache[reps]


def make_in_maps(x, w_qkv, w_out):
    """Host-side sharding: per-core input dicts (head-sharded)."""
    import ml_dtypes

    x = np.ascontiguousarray(np.asarray(x, dtype=np.float32))
    w_qkv = np.ascontiguousarray(np.asarray(w_qkv, dtype=np.float32))
    w_out = np.ascontiguousarray(np.asarray(w_out, dtype=np.float32))
    D = D_FULL
    bf = ml_dtypes.bfloat16
    xt = np.ascontiguousarray(x.transpose(0, 2, 1).astype(bf))
    cos_t, sin_t = host_tables(S_FULL)
    maskp = host_masks()
    ones = np.ones((128, 128), np.float32)
    in_maps = []
    for c in range(N_CORES):
        e0, e1 = c * HC_FULL * 128, (c + 1) * HC_FULL * 128
        in_maps.append(
            dict(
                xt=xt,
                wqt=np.ascontiguousarray(w_qkv[e0:e1].T.astype(bf)),
                wkt=np.ascontiguousarray(w_qkv[D + e0 : D + e1].T.astype(bf)),
                wvt=np.ascontiguousarray(w_qkv[2 * D + e0 : 2 * D + e1].T.astype(bf)),
                wot=np.ascontiguousarray(w_out[:, e0:e1].T),
                cost=cos_t,
                sint=sin_t,
                maskp=maskp,
                ones=ones,
            )
        )
    return in_maps


def combine(parts):
    """Sum the 8 per-core out-projection partials."""
    acc = np.zeros((B_FULL, S_FULL, D_FULL), np.float64)
    for p in parts:
        acc += p
    return acc.astype(np.float32)


def kernel(x, w_qkv, w_out):
    from concourse import bass_utils

    nc = get_compiled(reps=1)
    in_maps = make_in_maps(x, w_qkv, w_out)
    res = bass_utils.run_bass_kernel_spmd(
        nc, in_maps, core_ids=list(range(N_CORES))
    )
    return combine([res.results[c]["part"] for c in range(N_CORES)])


# revision 40
# speedup vs baseline: 1.1893x; 1.0005x over previous
import sys as _sys
import os as _os

for _p in ("/opt/trn_rl_repo", _os.path.expanduser("~/.axon_site/_ro/trn_rl_repo")):
    if _os.path.isdir(_p) and _p not in _sys.path:
        _sys.path.append(_p)

"""Builder for the sliding-window attention kernel (NaiveHybridAttention).

Per-core program (SPMD, head-sharded):
  inputs (per core): xT (B,D,S), wqT/wkT/wvT (D,E), woT (E,D),
                     cos/sin RoPE tables (HD,S), pair window masks (128,1024),
                     ones (128,128)
  output: part (B,S,D) = this core's heads' contribution to the final
          out-projection; host sums the 8 partials.

Pipeline per batch:
  A) QKV: qT,kT = W^T-stationary matmuls -> [e, S]; PSUM banks are released
     by a single ACT copy, RoPE runs on DVE from the SBUF copy; v =
     x-stationary -> [s, e].
  B) Attention per head, TRANSPOSED-scores dataflow: for each 256-query
     block, scoresT [k,q] come from kT-tile-stationary matmuls (k on
     partitions) so exp'd probs feed AV directly with NO PE transposes.
     Two adjacent k-tiles share one PSUM bank -> one [128,512] exp per
     pair; the (256,128)-offset pair is fully inside the window (no mask).
     Row-sums via a ones-column matmul; 1/rowsum is computed AFTER an
     outer-product broadcast (full-width DVE reciprocal) and folded into
     the AV-psum evacuation multiply.
  C) Out-proj: attnT-stationary -> psum [s, o] -> DMA to part on the SWDGE
     queue (keeps the HWDGE queue free for the next batch's x stream).

All matmuls run as float32r (full fp32 storage; 1 cycle/row at N>=256).
PSUM: 8 tagged bank slots: S0-S3 (score pairs / qkv q,k / outproj),
AV0-AV2 (AV rotation / qkv v), RS (rowsum+broadcast / qkv v).
"""

import os

import numpy as np
import concourse.bass as bass
from concourse import mybir

F32 = mybir.dt.float32
F32R = mybir.dt.float32r
BF16 = mybir.dt.bfloat16
ROPE_BASE = 10000.0
WINDOW = 512


def r32(ap):
    return ap.bitcast(F32R)


def host_tables(S, HD=128):
    """cos/sin tables in transposed layout [HD, S]. The sin table is
    PARTITION-SWAPPED and sign-folded (rows 0:64 = +sin, rows 64:128 = -sin)
    so each RoPE rot-multiply reads both SBUF inputs from the SAME partition
    range: rot[64:128] = q[0:64]*sin2[0:64], rot[0:64] = q[64:128]*sin2[64:128].
    Unscaled — the softmax 1/sqrt(HD) is applied via the Exp activation's
    scale parameter."""
    inv_freq = 1.0 / (ROPE_BASE ** (np.arange(0, HD, 2, dtype=np.float64) / HD))
    fr = np.arange(S, dtype=np.float64)[None, :] * inv_freq[:, None]  # [HD/2, S]
    cos = np.cos(fr)
    sin = np.sin(fr)
    cos_t = np.concatenate([cos, cos], 0).astype(np.float32)
    sin_sw = np.concatenate([sin, -sin], 0).astype(np.float32)
    return cos_t, sin_sw


def host_masks():
    """Multiplicative (1.0/0.0) sliding-window pair masks in the transposed
    [k, q] orientation. A score tile with offset d0 = q0 - ktile_start is
    valid where 0 <= d0 + qi - ki < WINDOW. Pattern A = tiles (d0=512|384),
    pattern B = tiles (d0=0|-128); the (256|128) pair is fully valid."""

    def m(d0):
        ki = np.arange(128)[:, None]
        qi = np.arange(256)[None, :]
        return ((d0 + qi - ki >= 0) & (d0 + qi - ki < WINDOW)).astype(np.float32)

    pa = np.concatenate([m(512), m(384)], axis=1)   # [128, 512]
    pb = np.concatenate([m(0), m(-128)], axis=1)    # [128, 512]
    return np.concatenate([pa, pb], axis=1)          # [128, 1024]


def partial_ref_np(x, wq_r, wk_r, wv_r, wo_t):
    """NumPy mirror of the per-core computation (fp32).
    x: (B,S,D); wq_r/wk_r/wv_r: (E,D) row-slices of w_qkv; wo_t: (E,D) =
    w_out[:, e_slice].T. Returns (B,S,D) partial."""
    B, S, D = x.shape
    E = wq_r.shape[0]
    HC = E // 128
    q = np.einsum("bsd,ed->bse", x, wq_r).reshape(B, S, HC, 128)
    k = np.einsum("bsd,ed->bse", x, wk_r).reshape(B, S, HC, 128)
    v = np.einsum("bsd,ed->bse", x, wv_r).reshape(B, S, HC, 128)
    inv_freq = 1.0 / (ROPE_BASE ** (np.arange(0, 128, 2, dtype=np.float64) / 128))
    fr = np.arange(S, dtype=np.float64)[:, None] * inv_freq[None, :]
    emb = np.concatenate([fr, fr], -1)
    cos = np.cos(emb).astype(np.float32)[None, :, None, :]
    sin = np.sin(emb).astype(np.float32)[None, :, None, :]

    def rot(t):
        t1, t2 = t[..., :64], t[..., 64:]
        return np.concatenate([-t2, t1], -1)

    q = q * cos + rot(q) * sin
    k = k * cos + rot(k) * sin
    scale = 1.0 / np.sqrt(128.0)
    i = np.arange(S)[:, None]
    j = np.arange(S)[None, :]
    valid = (i - j >= 0) & (i - j < WINDOW)
    out = np.zeros((B, S, E), np.float32)
    for b in range(B):
        for h in range(HC):
            s = (q[b, :, h] @ k[b, :, h].T) * scale
            s = np.where(valid, s, -np.inf)
            s = s - s.max(-1, keepdims=True)
            p = np.exp(s)
            p /= p.sum(-1, keepdims=True)
            out[b, :, h * 128 : (h + 1) * 128] = p @ v[b, :, h]
    return np.einsum("bse,ed->bsd", out, wo_t).astype(np.float32)


def declare_io(nc, B, S, D, E):
    dt = F32
    t = {}
    # x and the qkv weights stream in bf16: halves the dominant input-DMA
    # stream; accumulation stays fp32 in PSUM
    t["xt"] = nc.dram_tensor("xt", [B, D, S], BF16, kind="ExternalInput").ap()
    for n in ("wqt", "wkt", "wvt"):
        t[n] = nc.dram_tensor(n, [D, E], BF16, kind="ExternalInput").ap()
    t["wot"] = nc.dram_tensor("wot", [E, D], BF16, kind="ExternalInput").ap()
    for n in ("cost", "sint"):
        t[n] = nc.dram_tensor(n, [128, S], dt, kind="ExternalInput").ap()
    t["maskp"] = nc.dram_tensor("maskp", [128, 1024], BF16, kind="ExternalInput").ap()
    t["ones"] = nc.dram_tensor("ones", [128, 128], BF16, kind="ExternalInput").ap()
    # partial written bf16: halves the dominant output-DMA stream; the 8
    # per-core partials are summed in float64 on the host
    t["part"] = nc.dram_tensor("part", [B, S, D], BF16, kind="ExternalOutput").ap()
    return t


def build_program(ctx, nc, tc, io, B, S, D, HC, reps=1):
    """Emit the per-core program. HC = heads on this core; E = HC*128.
    reps > 1 wraps the body in a hardware loop repeating the identical
    computation (for timing measurements); output is unchanged."""
    E = HC * 128
    KT = D // 128  # contraction tiles for qkv

    const = ctx.enter_context(tc.tile_pool(name="const", bufs=1))
    work = ctx.enter_context(tc.tile_pool(name="work", bufs=1))
    xsp = ctx.enter_context(tc.tile_pool(name="xs", bufs=5))
    tmp = ctx.enter_context(tc.tile_pool(name="tmp", bufs=2))
    pp = ctx.enter_context(tc.tile_pool(name="pp", bufs=4))
    rbp = ctx.enter_context(tc.tile_pool(name="rb", bufs=2))
    outp = ctx.enter_context(tc.tile_pool(name="outp", bufs=3))
    ps = ctx.enter_context(tc.tile_pool(name="ps", bufs=1, space="PSUM"))

    # ---- constants ----
    # q/k/v weights: one DMA per 128-row k-tile so the first matmuls only
    # depend on the slices they read (kills the startup stall). Other consts
    # go on the gpsimd (SWDGE) queue to stay off the HWDGE queue that
    # streams x.
    wq_sb = const.tile([128, KT, E], BF16)
    wk_sb = const.tile([128, KT, E], BF16)
    wv_sb = const.tile([128, KT, E], BF16)
    for kt in range(KT):
        rows = bass.ts(kt, 128)
        nc.gpsimd.dma_start(wq_sb[:, kt, :], io["wqt"][rows, :])
        nc.gpsimd.dma_start(wk_sb[:, kt, :], io["wkt"][rows, :])
        nc.gpsimd.dma_start(wv_sb[:, kt, :], io["wvt"][rows, :])
    wo_sb = const.tile([128, HC, D], BF16)
    nc.gpsimd.dma_start(wo_sb[:], io["wot"].rearrange("(et p) o -> p et o", p=128))
    cost = const.tile([128, S], F32)
    nc.gpsimd.dma_start(cost[:], io["cost"][:])
    sint = const.tile([128, S], F32)
    nc.gpsimd.dma_start(sint[:], io["sint"][:])
    maskp = const.tile([128, 1024], BF16)
    nc.gpsimd.dma_start(maskp[:], io["maskp"][:])
    ones_sb = const.tile([128, 128], BF16)
    nc.gpsimd.dma_start(ones_sb[:], io["ones"][:])

    def rope_evac(src_ps, w, slot):
        """Release a q/k PSUM bank with a single DVE copy (first in the DVE
        queue for the chunk, so the bank frees before the next chunk's
        matmuls need it). Returns the SBUF copy."""
        sb = tmp.tile([128, 512], F32, tag=f"pcp{slot}")
        nc.vector.tensor_copy(sb[:, :w], src_ps[:, :w])
        return sb

    def rope(dst, sb, cos_t, sin_t, cols, w):
        """dst[:, cols] = sb*cos + swap_halves(sb)*sin (RoPE), from the SBUF
        copy made by rope_evac. The cos product lands in dst; rot is added
        in place."""
        rot = tmp.tile([128, 512], F32, tag="rot")
        nc.vector.tensor_mul(rot[0:64, :w], sb[64:128, :w], sin_t[64:128, cols])
        nc.vector.tensor_mul(rot[64:128, :w], sb[0:64, :w], sin_t[0:64, cols])
        nc.vector.tensor_mul(dst, sb[:, :w], cos_t[:, cols])
        nc.vector.tensor_add(dst, dst, rot[:, :w])

    def body():
        _emit_body(nc, tc, io, B, S, D, HC, locals_=dict(
            const=const, work=work, xsp=xsp, tmp=tmp, pp=pp,
            rbp=rbp, outp=outp, ps=ps,
            wq_sb=wq_sb, wk_sb=wk_sb, wv_sb=wv_sb, wo_sb=wo_sb,
            cost=cost, sint=sint, maskp=maskp, ones_sb=ones_sb, rope=rope,
            rope_evac=rope_evac,
        ))

    if reps > 1:
        with tc.For_i(0, reps, 1):
            body()
    else:
        body()


def _emit_body(nc, tc, io, B, S, D, HC, locals_):
    E = HC * 128
    KT = D // 128
    SC = S // 512
    QB = S // 256
    ST = S // 128
    OCW = min(512, D)
    OC = D // OCW
    work = locals_["work"]; xsp = locals_["xsp"]
    pp = locals_["pp"]; rbp = locals_["rbp"]
    outp = locals_["outp"]; ps = locals_["ps"]
    wq_sb = locals_["wq_sb"]; wk_sb = locals_["wk_sb"]; wv_sb = locals_["wv_sb"]
    wo_sb = locals_["wo_sb"]; cost = locals_["cost"]; sint = locals_["sint"]
    maskp = locals_["maskp"]; ones_sb = locals_["ones_sb"]
    rope = locals_["rope"]
    rope_evac = locals_["rope_evac"]

    for b in range(B):
        # ---- A) QKV projection ----
        qT = work.tile([128, HC, S], BF16, tag="qT")
        kT = work.tile([128, HC, S], BF16, tag="kT")
        v_sb = work.tile([128, ST, E], BF16, tag="v")
        for sc in range(SC):
            cols = bass.ts(sc, 512)
            q_ps = [
                ps.tile([128, 512], F32, tag=t, name=f"q_ps{i}")
                for i, t in enumerate(("S0", "S1")[:HC])
            ]
            k_ps = [
                ps.tile([128, 512], F32, tag=t, name=f"k_ps{i}")
                for i, t in enumerate(("S2", "S3")[:HC])
            ]
            v_ps = [
                ps.tile([128, E], F32, tag=t, name=f"v_ps{i}")
                for i, t in enumerate(("AV0", "AV1", "AV2", "RS"))
            ]
            for kt in range(KT):
                xs = xsp.tile([128, 512], BF16)
                nc.sync.dma_start(xs[:], io["xt"][b, bass.ts(kt, 128), cols])
                f = dict(start=(kt == 0), stop=(kt == KT - 1))
                # v first: its psum slots are evacuated fastest, so the next
                # chunk's accumulation can begin while q/k RoPE evac runs
                for ss in range(4):
                    nc.tensor.matmul(
                        v_ps[ss][:],
                        xs[:, bass.ts(ss, 128)],
                        wv_sb[:, kt, :],
                        **f,
                    )
                for et in range(HC):
                    nc.tensor.matmul(
                        k_ps[et][:], wk_sb[:, kt, bass.ts(et, 128)], xs[:], **f
                    )
                    nc.tensor.matmul(
                        q_ps[et][:], wq_sb[:, kt, bass.ts(et, 128)], xs[:], **f
                    )
            # v-bank releases on ACT (first in its queue), q/k-bank releases
            # on DVE, both in next-chunk consumption order (v, then k,q per
            # head); rope math follows from the SBUF copies
            for ss in range(4):
                nc.scalar.copy(v_sb[:, sc * 4 + ss, :], v_ps[ss][:])
            sbs = []
            for et in range(HC):
                sbk = rope_evac(k_ps[et], 512, 2 * et)
                sbq = rope_evac(q_ps[et], 512, 2 * et + 1)
                sbs.append((sbk, sbq))
            for et in range(HC):
                sbk, sbq = sbs[et]
                rope(kT[:, et, cols], sbk, cost, sint, cols, 512)
                rope(qT[:, et, cols], sbq, cost, sint, cols, 512)

        # ---- B) attention, per head (transposed-scores dataflow) ----
        # Per 256-query block: up to 3 PAIRS of scoresT [k,q] tiles, each
        # pair filling one [128,512] PSUM bank (2 matmuls), one exp per
        # pair on ACT, pair mask on DVE (middle pair needs none),
        # ones-matmul row-sums + AV on PE, normalization via broadcast +
        # full-width reciprocal folded into the AV evacuation multiply.
        attnT = work.tile([128, HC, S], BF16, tag="attnT")
        exp_scale = float(1.0 / np.sqrt(128.0))
        state = dict(gidx=0, blk=0)

        def tile_cols(d):
            """Valid query-column range (offset, len) for a score tile with
            offset d: d=512 touches only the first 128 queries, d=-128 only
            the last 128 (the rest is fully outside the window)."""
            if d == 512:
                return 0, 128
            if d == -128:
                return 128, 128
            return 0, 256

        def emit_front(h, qb):
            """Score matmuls + exp + mask for block (h, qb)."""
            q0 = qb * 256
            kstart = max(0, q0 - WINDOW)
            nkt = (q0 + 256 - kstart) // 128
            ptiles = []
            for pr in range(nkt // 2):
                kt0 = 2 * pr
                d0 = q0 - kstart - 128 * kt0
                sp = ps.tile(
                    [128, 512], F32, tag=f"S{state['gidx'] % 4}", name="sp"
                )
                state["gidx"] += 1
                for t in range(2):
                    off, ln = tile_cols(d0 - 128 * t)
                    nc.tensor.matmul(
                        sp[:, bass.ds(256 * t + off, ln)],
                        kT[:, h, bass.ds(kstart + 128 * (kt0 + t), 128)],
                        qT[:, h, bass.ds(q0 + off, ln)],
                    )
                p_sb = pp.tile([128, 512], BF16, tag=f"p{pr}", name=f"p{pr}")
                nc.scalar.activation(
                    p_sb[:], sp[:], mybir.ActivationFunctionType.Exp,
                    scale=exp_scale,
                )
                # mask zeroes everything outside the window, including the
                # skipped half-tiles' columns (whose exp read stale psum and
                # is never consumed downstream)
                if d0 == 512:
                    nc.vector.tensor_mul(p_sb[:], p_sb[:], maskp[:, 0:512])
                elif d0 == 0:
                    nc.vector.tensor_mul(p_sb[:], p_sb[:], maskp[:, 512:1024])
                # d0 == 256: pair fully inside the window, no mask
                ptiles.append(p_sb)
            return dict(h=h, q0=q0, kstart=kstart, nkt=nkt, p=ptiles)

        def emit_tail(d):
            """Row-sums, AV, broadcast-normalize for a previously-issued
            block."""
            h, q0, kstart, nkt = d["h"], d["q0"], d["kstart"], d["nkt"]
            avx = state["blk"] % 3
            state["blk"] += 1
            d0b = q0 - kstart
            rs = ps.tile([1, 256], F32, tag="RS", name="rs")
            for kt in range(nkt):
                off, ln = tile_cols(d0b - 128 * kt)
                nc.tensor.matmul(
                    rs[:, bass.ds(off, ln)], ones_sb[:, 0:1],
                    d["p"][kt // 2][:, bass.ds(256 * (kt % 2) + off, ln)],
                    start=(kt == 0), stop=(kt == nkt - 1),
                )
            rs_sb = rbp.tile([1, 256], BF16, tag="rss")
            nc.vector.tensor_copy(rs_sb[:], rs[:])
            av = ps.tile([128, 256], F32, tag=f"AV{avx}", name="av")
            for kt in range(nkt):
                off, ln = tile_cols(d0b - 128 * kt)
                nc.tensor.matmul(
                    av[:, bass.ds(off, ln)],
                    v_sb[:, kstart // 128 + kt, bass.ts(h, 128)],
                    d["p"][kt // 2][:, bass.ds(256 * (kt % 2) + off, ln)],
                    start=(kt == 0), stop=(kt == nkt - 1),
                )
            rb_ps = ps.tile([128, 256], F32, tag="RS", name="rb")
            nc.tensor.matmul(rb_ps[:], ones_sb[0:1, :], rs_sb[:])
            rb_sb = rbp.tile([128, 256], F32, tag="rbs")
            nc.vector.reciprocal(rb_sb[:], rb_ps[:])
            nc.vector.tensor_mul(attnT[:, h, bass.ds(q0, 256)], av[:], rb_sb[:])

        # three-stage software pipeline: scores+exp run three blocks ahead
        # of rowsum/AV/normalize, so PE streams through the ACT/DVE softmax
        # chain even across the phase-entry DVE queue drain
        from collections import deque

        pend = deque()
        for h in range(HC):
            for qb in range(QB):
                pend.append(emit_front(h, qb))
                if len(pend) > 3:
                    emit_tail(pend.popleft())
        while pend:
            emit_tail(pend.popleft())

        # ---- C) out-projection (partial over this core's E dims) ----
        # evac copies alternate DVE/ACT; one batched 1 MB output DMA per
        # 128-row stripe, on the SWDGE queue so the HWDGE queue stays free
        # for the next batch's x stream
        for st in range(ST):
            osb = outp.tile([128, D], BF16, tag="osb")
            for oc in range(OC):
                o_ps = ps.tile(
                    [128, OCW], F32, tag=f"S{oc % 4}", name="o_ps"
                )
                for et in range(HC):
                    nc.tensor.matmul(
                        o_ps[:],
                        attnT[:, et, bass.ts(st, 128)],
                        wo_sb[:, et, bass.ts(oc, OCW)],
                        start=(et == 0),
                        stop=(et == HC - 1),
                    )
                dst = osb[:, bass.ts(oc, OCW)]
                if oc % 2 == 0:
                    nc.vector.tensor_copy(dst, o_ps[:])
                else:
                    nc.scalar.copy(dst, o_ps[:])
            nc.gpsimd.dma_start(io["part"][b, bass.ts(st, 128), :], osb[:])


# ======================================================================
# 8-core SPMD wrapper
# ======================================================================
from contextlib import ExitStack as _ExitStack

N_CORES = 8
B_FULL, S_FULL, D_FULL, H_FULL, HD_FULL = 2, 2048, 2048, 16, 128
HC_FULL = H_FULL // N_CORES  # 2 heads per core

_nc_cache = {}


def get_compiled(reps=1):
    """Build + bacc-compile the per-core Bass program (cached per reps)."""
    if reps not in _nc_cache:
        import concourse.bacc as bacc
        from concourse import tile

        nc = bacc.Bacc(
            "TRN2", target_bir_lowering=False, debug=False, num_devices=N_CORES
        )
        io = declare_io(nc, B_FULL, S_FULL, D_FULL, HC_FULL * 128)
        with tile.TileContext(nc) as tc:
            with _ExitStack() as ctx:
                build_program(
                    ctx, nc, tc, io, B_FULL, S_FULL, D_FULL, HC_FULL, reps=reps
                )
        nc.compile()
        _nc_cache[reps] = nc
    return _nc_cache[reps]


def make_in_maps(x, w_qkv, w_out):
    """Host-side sharding: per-core input dicts (head-sharded)."""
    import ml_dtypes

    x = np.ascontiguousarray(np.asarray(x, dtype=np.float32))
    w_qkv = np.ascontiguousarray(np.asarray(w_qkv, dtype=np.float32))
    w_out = np.ascontiguousarray(np.asarray(w_out, dtype=np.float32))
    D = D_FULL
    bf = ml_dtypes.bfloat16
    xt = np.ascontiguousarray(x.transpose(0, 2, 1).astype(bf))
    cos_t, sin_t = host_tables(S_FULL)
    maskp = host_masks().astype(bf)
    ones = np.ones((128, 128), bf)
    in_maps = []
    for c in range(N_CORES):
        e0, e1 = c * HC_FULL * 128, (c + 1) * HC_FULL * 128
        in_maps.append(
            dict(
                xt=xt,
                wqt=np.ascontiguousarray(w_qkv[e0:e1].T.astype(bf)),
                wkt=np.ascontiguousarray(w_qkv[D + e0 : D + e1].T.astype(bf)),
                wvt=np.ascontiguousarray(w_qkv[2 * D + e0 : 2 * D + e1].T.astype(bf)),
                wot=np.ascontiguousarray(w_out[:, e0:e1].T.astype(bf)),
                cost=cos_t,
                sint=sin_t,
                maskp=maskp,
                ones=ones,
            )
        )
    return in_maps


def combine(parts):
    """Sum the 8 per-core out-projection partials."""
    acc = np.zeros((B_FULL, S_FULL, D_FULL), np.float64)
    for p in parts:
        acc += p
    return acc.astype(np.float32)


def kernel(x, w_qkv, w_out):
    from concourse import bass_utils

    nc = get_compiled(reps=1)
    in_maps = make_in_maps(x, w_qkv, w_out)
    res = bass_utils.run_bass_kernel_spmd(
        nc, in_maps, core_ids=list(range(N_CORES))
    )
    return combine([res.results[c]["part"] for c in range(N_CORES)])
